# revision 21
# baseline (speedup 1.0000x reference)
"""Trainium2 Bass kernel for ConditionalLoRALinear.

Reference computation (f32):
    base = x @ W.T + b                      # [B,S,Do]
    lora = (x @ A.T) @ B.T * 2.0            # rank-8
    out  = base + lora * (ids == 7)         # per-token gate

Sharding over 8 NeuronCores: 2 token-halves x 4 d_out-quarters.
All matmul operands are bf16 (f32 PSUM accumulation), which runs the PE
at 1 cycle/row for any output width and halves HBM traffic vs f32r.

The rank-8 activations are folded into the main matmul: each k-chunk's
moving operand is [W_chunk | A_chunk] (128 x 1032), issued as three
344-wide matmuls into three PSUM banks, so xa = x @ A.T accumulates in
the last 8 columns of the third bank for free (no separate small matmul,
and every matmul is long enough to hide its weight load).  The epilogue
gates xa by the host-precomputed {0,2} mask, transposes it to [8, tok]
on the PE, and runs the rank-8 update as three matmuls that accumulate
into the same PSUM banks as the base result.  Bias is added during the
PSUM->SBUF copy on DVE.
"""

import sys

for _p in ("/opt/trn_rl_repo",):
    if _p not in sys.path:
        sys.path.insert(0, _p)

from contextlib import ExitStack

import numpy as np
import ml_dtypes

import concourse.bass as bass
import concourse.mybir as mybir
import concourse.tile as tile
from concourse import bacc
from concourse.bass import ts
from concourse.bass_utils import run_bass_kernel_spmd

F32 = mybir.dt.float32
BF16 = mybir.dt.bfloat16
BF = ml_dtypes.bfloat16

B, S, DI, DO = 4, 4096, 4096, 4096
TOK = B * S              # 16384 tokens
NCORES = 8
TH = TOK // 2            # tokens per core (half)        = 8192
DQ = DO // 4             # d_out per core (quarter)      = 1024
P = 128                  # partition / strip size
KC = DI // P             # k-chunks                      = 32
NSTRIP = TH // P         # token strips per core         = 64
R = 8                    # LoRA rank
WAC = DQ + R             # combined W+A columns          = 1032
NQ = 3                   # output chunks per strip
CW = WAC // NQ           # chunk width                   = 344
WB = 4                   # k-chunks per W-tile DMA
COMP_TOKEN_ID = 7
SCALING = 2.0


def _build_nc():
    nc = bacc.Bacc(
        "TRN2",
        target_bir_lowering=False,
        debug=False,
        enable_asserts=True,
        num_devices=NCORES,
    )

    xT_d = nc.dram_tensor("xT", [NSTRIP, P, KC * P], BF16, kind="ExternalInput").ap()
    # W+A tiles pre-gathered on host into per-partition-contiguous layout so
    # each preload DMA is one 8 KB contiguous run per partition (the strided
    # (b p) o -> p b o gather pattern runs the DMA queue at ~half rate and
    # delayed W arrival, gating the first ~30 us of the kernel).
    wa_d = nc.dram_tensor(
        "wa", [KC // WB, P, WB * WAC], BF16, kind="ExternalInput"
    ).ap()
    bT_d = nc.dram_tensor("bT", [R, DQ], BF16, kind="ExternalInput").ap()
    bias_d = nc.dram_tensor("biasr", [P, DQ], F32, kind="ExternalInput").ap()
    mask_d = nc.dram_tensor("maskp", [P, NSTRIP], F32, kind="ExternalInput").ap()
    id_d = nc.dram_tensor("ident", [P, P], BF16, kind="ExternalInput").ap()
    out_d = nc.dram_tensor("out", [TH, DQ], F32, kind="ExternalOutput").ap()

    with tile.TileContext(nc) as tc, ExitStack() as ctx:
        consts = ctx.enter_context(tc.tile_pool(name="consts", bufs=1))
        xpool = ctx.enter_context(tc.tile_pool(name="xp", bufs=2))
        opool = ctx.enter_context(tc.tile_pool(name="op", bufs=2))
        spool = ctx.enter_context(tc.tile_pool(name="sp", bufs=2))
        psum = ctx.enter_context(tc.tile_pool(name="ps", bufs=2, space="PSUM"))

        # ---- resident constants ----
        # W chunks alternate between the two descriptor-generation paths
        # (sync=HWDGE, gpsimd=SWDGE) so the initial load isn't serialized
        # on a single queue ahead of the first compute.
        bt = consts.tile([R, DQ], BF16, name="bt", tag="bt")
        nc.sync.dma_start(bt[:], bT_d[:, :])
        maskt = consts.tile([P, NSTRIP], F32, name="maskt", tag="maskt")
        nc.sync.dma_start(maskt[:], mask_d[:, :])
        ident = consts.tile([P, P], BF16, name="ident", tag="ident")
        nc.sync.dma_start(ident[:], id_d[:, :])
        wa_tiles = []
        for wb in range(KC // WB):
            wt = consts.tile([P, WB, WAC], BF16, name=f"w{wb}", tag=f"w{wb}")
            eng = nc.sync if wb % 2 == 0 else nc.gpsimd
            eng.dma_start(wt[:], wa_d[wb].rearrange("p (b o) -> p b o", o=WAC))
            wa_tiles.append(wt)
        biast = consts.tile([P, DQ], F32, name="biast", tag="biast")
        nc.sync.dma_start(biast[:], bias_d[:, :])

        def cw(j):
            """output width of chunk j (last chunk excludes the xa columns)."""
            return CW if j < NQ - 1 else CW - R

        def epi1(st):
            """gate the strip's rank-8 activations and transpose to [r, tok]."""
            s, q = st
            xag = spool.tile([P, R], BF16, name="xag", tag="xag")
            nc.vector.tensor_scalar_mul(
                xag[:], q[NQ - 1][:, CW - R : CW], maskt[:, s : s + 1]
            )
            xaT_ps = psum.tile([R, P], BF16, name="xaT_ps", tag="pxat")
            nc.tensor.transpose(xaT_ps[:], xag[:], ident[:])
            xaT = spool.tile([R, P], BF16, name="xaT", tag="xaT")
            nc.vector.tensor_copy(xaT[:], xaT_ps[:])
            return xaT

        def epi2(st, xaT):
            """rank-8 update accumulated into the strip's own PSUM banks."""
            s, q = st
            for j in range(NQ):
                w = cw(j)
                nc.tensor.matmul(
                    q[j][:, 0:w],
                    xaT[:],
                    bt[:, j * CW : j * CW + w],
                    start=False,
                    stop=True,
                    skip_group_check=True,
                )

        def epi3(st):
            """bias add on DVE during PSUM->SBUF copy, then store."""
            s, q = st
            ob = opool.tile([P, DQ], F32, name="ob", tag="ob")
            for j in range(NQ):
                w = cw(j)
                sl = slice(j * CW, j * CW + w)
                nc.vector.tensor_add(ob[:, sl], q[j][:, 0:w], biast[:, sl])
            nc.sync.dma_start(out_d[ts(s, P), :], ob[:])

        prev = None
        prev_xaT = None
        E1, E2, E3 = 2, 6, 10
        for s in range(NSTRIP):
            xt = xpool.tile([P, KC, P], BF16, name="xt", tag="xt")
            nc.scalar.dma_start(xt[:], xT_d[s].rearrange("p (c t) -> p c t", t=P))
            q = [
                psum.tile([P, CW], F32, name=f"q{j}", tag=f"q{j}") for j in range(NQ)
            ]
            for c in range(KC):
                # previous strip's epilogue mid-stream: its PSUM slots are
                # released well before the next strip needs them, so the PE
                # never idles across a strip boundary.
                if prev is not None:
                    if c == E1:
                        prev_xaT = epi1(prev)
                    elif c == E2:
                        epi2(prev, prev_xaT)
                    elif c == E3:
                        epi3(prev)
                        prev = None
                lhsT = xt[:, c, :]
                for j in range(NQ):
                    nc.tensor.matmul(
                        q[j][:],
                        lhsT,
                        wa_tiles[c // WB][:, c % WB, ts(j, CW)],
                        start=(c == 0),
                        stop=False,
                        skip_group_check=True,
                    )
            prev = (s, q)

        prev_xaT = epi1(prev)
        epi2(prev, prev_xaT)
        epi3(prev)

    nc.compile()
    return nc


_NC_CACHE = None


def _get_nc():
    global _NC_CACHE
    if _NC_CACHE is None:
        _NC_CACHE = _build_nc()
    return _NC_CACHE


def _make_in_maps(x, ids, W, b, lora_A, lora_B):
    x2 = np.asarray(x, dtype=np.float32).reshape(TOK, DI).astype(BF)
    xT = np.ascontiguousarray(x2.T)                          # [DI, TOK] bf16
    WT = np.asarray(W, dtype=np.float32).T.astype(BF)        # [DI, DO]
    AT = np.asarray(lora_A, dtype=np.float32).T.astype(BF)   # [DI, R]
    BT = np.asarray(lora_B, dtype=np.float32).T.astype(BF)   # [R, DO]
    bias = np.asarray(b, dtype=np.float32)
    maskf = (np.asarray(ids).reshape(TOK) == COMP_TOKEN_ID).astype(
        np.float32
    ) * SCALING

    # strip-contiguous layout: xprep[s, p, c*128+t] = x[h*TH + s*128+t, c*128+p]
    xT_half = [
        np.ascontiguousarray(
            xT[:, h * TH : (h + 1) * TH]
            .reshape(KC, P, NSTRIP, P)
            .transpose(2, 1, 0, 3)
            .reshape(NSTRIP, P, KC * P)
        )
        for h in range(2)
    ]
    mask_half = [
        np.ascontiguousarray(maskf[h * TH : (h + 1) * TH].reshape(NSTRIP, P).T)
        for h in range(2)
    ]
    # per-quarter [W_chunk | A_chunk] combined moving operand, pre-gathered
    # into the SBUF tile layout: wa2[wb, p, b*WAC+o] = WA[(wb*WB+b)*128+p, o]
    ATr = np.asarray(AT).reshape(KC, P, R)
    wa_q = []
    for qi in range(4):
        WTq = WT[:, qi * DQ : (qi + 1) * DQ].reshape(KC, P, DQ)
        wa = np.concatenate([WTq, ATr], axis=2)  # [KC, P, WAC]
        wa_q.append(
            np.ascontiguousarray(
                wa.reshape(KC // WB, WB, P, WAC)
                .transpose(0, 2, 1, 3)
                .reshape(KC // WB, P, WB * WAC)
            )
        )
    bT_q = [
        np.ascontiguousarray(BT[:, qi * DQ : (qi + 1) * DQ]) for qi in range(4)
    ]
    bias_q = [
        np.ascontiguousarray(np.broadcast_to(bias[qi * DQ : (qi + 1) * DQ], (P, DQ)))
        for qi in range(4)
    ]
    ident = np.eye(P, dtype=np.float32).astype(BF)

    in_maps = []
    for c in range(NCORES):
        h, qi = c // 4, c % 4
        in_maps.append(
            {
                "xT": xT_half[h],
                "wa": wa_q[qi],
                "bT": bT_q[qi],
                "biasr": bias_q[qi],
                "maskp": mask_half[h],
                "ident": ident,
            }
        )
    return in_maps


def kernel(x, ids, W, b, lora_A, lora_B):
    nc = _get_nc()
    in_maps = _make_in_maps(x, ids, W, b, lora_A, lora_B)
    results = run_bass_kernel_spmd(nc, in_maps, core_ids=list(range(NCORES)))
    out = np.empty((TOK, DO), dtype=np.float32)
    for c in range(NCORES):
        h, qi = c // 4, c % 4
        out[h * TH : (h + 1) * TH, qi * DQ : (qi + 1) * DQ] = results.results[c]["out"]
    return out.reshape(B, S, DO)


if __name__ == "__main__":
    rng = np.random.default_rng(0)
    x = rng.standard_normal((B, S, DI), dtype=np.float32)
    ids = rng.integers(0, 64, size=(B, S)).astype(np.int64)
    W = rng.standard_normal((DO, DI), dtype=np.float32) / np.sqrt(DI)
    b = (rng.standard_normal(DO) * 0.02).astype(np.float32)
    lora_A = rng.standard_normal((8, DI), dtype=np.float32) / np.sqrt(DI)
    lora_B = (rng.standard_normal((DO, 8)) * 0.02).astype(np.float32)
    out = kernel(x, ids, W, b, lora_A, lora_B)
    print(out.shape, out.dtype, float(np.abs(out).mean()))


# revision 24
# speedup vs baseline: 1.0025x; 1.0025x over previous
"""Trainium2 Bass kernel for ConditionalLoRALinear.

Reference computation (f32):
    base = x @ W.T + b                      # [B,S,Do]
    lora = (x @ A.T) @ B.T * 2.0            # rank-8
    out  = base + lora * (ids == 7)         # per-token gate

Sharding over 8 NeuronCores: 2 token-halves x 4 d_out-quarters.
All matmul operands are bf16 (f32 PSUM accumulation), which runs the PE
at 1 cycle/row for any output width and halves HBM traffic vs f32r.

The rank-8 activations are folded into the main matmul: each k-chunk's
moving operand is [W_chunk | A_chunk] (128 x 1032), issued as three
344-wide matmuls into three PSUM banks, so xa = x @ A.T accumulates in
the last 8 columns of the third bank for free (no separate small matmul,
and every matmul is long enough to hide its weight load).  The epilogue
gates xa by the host-precomputed {0,2} mask, transposes it to [8, tok]
on the PE, and runs the rank-8 update as three matmuls that accumulate
into the same PSUM banks as the base result.  Bias is added during the
PSUM->SBUF copy on DVE.
"""

import sys

for _p in ("/opt/trn_rl_repo",):
    if _p not in sys.path:
        sys.path.insert(0, _p)

from contextlib import ExitStack

import numpy as np
import ml_dtypes

import concourse.bass as bass
import concourse.mybir as mybir
import concourse.tile as tile
from concourse import bacc
from concourse.bass import ts
from concourse.bass_utils import run_bass_kernel_spmd

F32 = mybir.dt.float32
BF16 = mybir.dt.bfloat16
BF = ml_dtypes.bfloat16

B, S, DI, DO = 4, 4096, 4096, 4096
TOK = B * S              # 16384 tokens
NCORES = 8
TH = TOK // 2            # tokens per core (half)        = 8192
DQ = DO // 4             # d_out per core (quarter)      = 1024
P = 128                  # partition / strip size
KC = DI // P             # k-chunks                      = 32
NSTRIP = TH // P         # token strips per core         = 64
R = 8                    # LoRA rank
WAC = DQ + R             # combined W+A columns          = 1032
NQ = 3                   # output chunks per strip
CW = WAC // NQ           # chunk width                   = 344
WB = 4                   # k-chunks per W-tile DMA
COMP_TOKEN_ID = 7
SCALING = 2.0


def _build_nc():
    nc = bacc.Bacc(
        "TRN2",
        target_bir_lowering=False,
        debug=False,
        enable_asserts=True,
        num_devices=NCORES,
    )

    xT_d = nc.dram_tensor("xT", [NSTRIP, P, KC * P], BF16, kind="ExternalInput").ap()
    wa_d = nc.dram_tensor("wa", [KC * P, WAC], BF16, kind="ExternalInput").ap()
    bT_d = nc.dram_tensor("bT", [R, DQ], BF16, kind="ExternalInput").ap()
    bias_d = nc.dram_tensor("biasr", [P, DQ], F32, kind="ExternalInput").ap()
    mask_d = nc.dram_tensor("maskp", [P, NSTRIP], F32, kind="ExternalInput").ap()
    id_d = nc.dram_tensor("ident", [P, P], BF16, kind="ExternalInput").ap()
    out_d = nc.dram_tensor("out", [TH, DQ], F32, kind="ExternalOutput").ap()

    with tile.TileContext(nc) as tc, ExitStack() as ctx:
        consts = ctx.enter_context(tc.tile_pool(name="consts", bufs=1))
        xpool = ctx.enter_context(tc.tile_pool(name="xp", bufs=2))
        opool = ctx.enter_context(tc.tile_pool(name="op", bufs=2))
        spool = ctx.enter_context(tc.tile_pool(name="sp", bufs=2))
        psum = ctx.enter_context(tc.tile_pool(name="ps", bufs=2, space="PSUM"))

        # ---- resident constants ----
        # W chunks alternate between the two descriptor-generation paths
        # (sync=HWDGE, gpsimd=SWDGE) so the initial load isn't serialized
        # on a single queue ahead of the first compute.
        bt = consts.tile([R, DQ], BF16, name="bt", tag="bt")
        nc.sync.dma_start(bt[:], bT_d[:, :])
        maskt = consts.tile([P, NSTRIP], F32, name="maskt", tag="maskt")
        nc.sync.dma_start(maskt[:], mask_d[:, :])
        ident = consts.tile([P, P], BF16, name="ident", tag="ident")
        nc.sync.dma_start(ident[:], id_d[:, :])
        wa_tiles = []
        for wb in range(KC // WB):
            wt = consts.tile([P, WB, WAC], BF16, name=f"w{wb}", tag=f"w{wb}")
            eng = nc.sync if wb % 2 == 0 else nc.gpsimd
            eng.dma_start(
                wt[:], wa_d[ts(wb, WB * P), :].rearrange("(b p) o -> p b o", p=P)
            )
            wa_tiles.append(wt)
        biast = consts.tile([P, DQ], F32, name="biast", tag="biast")
        nc.sync.dma_start(biast[:], bias_d[:, :])

        def cw(j):
            """output width of chunk j (last chunk excludes the xa columns)."""
            return CW if j < NQ - 1 else CW - R

        def epi1(st):
            """gate the strip's rank-8 activations and transpose to [r, tok]."""
            s, q = st
            xag = spool.tile([P, R], BF16, name="xag", tag="xag")
            nc.vector.tensor_scalar_mul(
                xag[:], q[NQ - 1][:, CW - R : CW], maskt[:, s : s + 1]
            )
            xaT_ps = psum.tile([R, P], BF16, name="xaT_ps", tag="pxat")
            nc.tensor.transpose(xaT_ps[:], xag[:], ident[:])
            xaT = spool.tile([R, P], BF16, name="xaT", tag="xaT")
            nc.vector.tensor_copy(xaT[:], xaT_ps[:])
            return xaT

        def epi2(st, xaT):
            """rank-8 update accumulated into the strip's own PSUM banks."""
            s, q = st
            for j in range(NQ):
                w = cw(j)
                nc.tensor.matmul(
                    q[j][:, 0:w],
                    xaT[:],
                    bt[:, j * CW : j * CW + w],
                    start=False,
                    stop=True,
                    skip_group_check=True,
                )

        def epi3(st):
            """bias add on DVE during PSUM->SBUF copy, then store."""
            s, q = st
            ob = opool.tile([P, DQ], F32, name="ob", tag="ob")
            for j in range(NQ):
                w = cw(j)
                sl = slice(j * CW, j * CW + w)
                nc.vector.tensor_add(ob[:, sl], q[j][:, 0:w], biast[:, sl])
            nc.sync.dma_start(out_d[ts(s, P), :], ob[:])

        prev = None
        prev_xaT = None
        E1, E2, E3 = 2, 6, 10
        for s in range(NSTRIP):
            xt = xpool.tile([P, KC, P], BF16, name="xt", tag="xt")
            nc.scalar.dma_start(xt[:], xT_d[s].rearrange("p (c t) -> p c t", t=P))
            q = [
                psum.tile([P, CW], F32, name=f"q{j}", tag=f"q{j}") for j in range(NQ)
            ]
            for c in range(KC):
                # previous strip's epilogue mid-stream: its PSUM slots are
                # released well before the next strip needs them, so the PE
                # never idles across a strip boundary.
                if prev is not None:
                    if c == E1:
                        prev_xaT = epi1(prev)
                    elif c == E2:
                        epi2(prev, prev_xaT)
                    elif c == E3:
                        epi3(prev)
                        prev = None
                lhsT = xt[:, c, :]
                for j in range(NQ):
                    nc.tensor.matmul(
                        q[j][:],
                        lhsT,
                        wa_tiles[c // WB][:, c % WB, ts(j, CW)],
                        start=(c == 0),
                        stop=False,
                        skip_group_check=True,
                    )
            prev = (s, q)

        prev_xaT = epi1(prev)
        epi2(prev, prev_xaT)
        epi3(prev)

    nc.compile()
    return nc


_NC_CACHE = None


def _get_nc():
    global _NC_CACHE
    if _NC_CACHE is None:
        _NC_CACHE = _build_nc()
    return _NC_CACHE


def _make_in_maps(x, ids, W, b, lora_A, lora_B):
    x2 = np.asarray(x, dtype=np.float32).reshape(TOK, DI).astype(BF)
    xT = np.ascontiguousarray(x2.T)                          # [DI, TOK] bf16
    WT = np.asarray(W, dtype=np.float32).T.astype(BF)        # [DI, DO]
    AT = np.asarray(lora_A, dtype=np.float32).T.astype(BF)   # [DI, R]
    BT = np.asarray(lora_B, dtype=np.float32).T.astype(BF)   # [R, DO]
    bias = np.asarray(b, dtype=np.float32)
    maskf = (np.asarray(ids).reshape(TOK) == COMP_TOKEN_ID).astype(
        np.float32
    ) * SCALING

    # strip-contiguous layout: xprep[s, p, c*128+t] = x[h*TH + s*128+t, c*128+p]
    xT_half = [
        np.ascontiguousarray(
            xT[:, h * TH : (h + 1) * TH]
            .reshape(KC, P, NSTRIP, P)
            .transpose(2, 1, 0, 3)
            .reshape(NSTRIP, P, KC * P)
        )
        for h in range(2)
    ]
    mask_half = [
        np.ascontiguousarray(maskf[h * TH : (h + 1) * TH].reshape(NSTRIP, P).T)
        for h in range(2)
    ]
    # per-quarter [W_chunk | A_chunk] combined moving operand
    ATr = np.asarray(AT).reshape(KC, P, R)
    wa_q = []
    for qi in range(4):
        WTq = WT[:, qi * DQ : (qi + 1) * DQ].reshape(KC, P, DQ)
        wa_q.append(
            np.ascontiguousarray(
                np.concatenate([WTq, ATr], axis=2).reshape(KC * P, WAC)
            )
        )
    bT_q = [
        np.ascontiguousarray(BT[:, qi * DQ : (qi + 1) * DQ]) for qi in range(4)
    ]
    bias_q = [
        np.ascontiguousarray(np.broadcast_to(bias[qi * DQ : (qi + 1) * DQ], (P, DQ)))
        for qi in range(4)
    ]
    ident = np.eye(P, dtype=np.float32).astype(BF)

    in_maps = []
    for c in range(NCORES):
        h, qi = c // 4, c % 4
        in_maps.append(
            {
                "xT": xT_half[h],
                "wa": wa_q[qi],
                "bT": bT_q[qi],
                "biasr": bias_q[qi],
                "maskp": mask_half[h],
                "ident": ident,
            }
        )
    return in_maps


def kernel(x, ids, W, b, lora_A, lora_B):
    nc = _get_nc()
    in_maps = _make_in_maps(x, ids, W, b, lora_A, lora_B)
    results = run_bass_kernel_spmd(nc, in_maps, core_ids=list(range(NCORES)))
    out = np.empty((TOK, DO), dtype=np.float32)
    for c in range(NCORES):
        h, qi = c // 4, c % 4
        out[h * TH : (h + 1) * TH, qi * DQ : (qi + 1) * DQ] = results.results[c]["out"]
    return out.reshape(B, S, DO)


if __name__ == "__main__":
    rng = np.random.default_rng(0)
    x = rng.standard_normal((B, S, DI), dtype=np.float32)
    ids = rng.integers(0, 64, size=(B, S)).astype(np.int64)
    W = rng.standard_normal((DO, DI), dtype=np.float32) / np.sqrt(DI)
    b = (rng.standard_normal(DO) * 0.02).astype(np.float32)
    lora_A = rng.standard_normal((8, DI), dtype=np.float32) / np.sqrt(DI)
    lora_B = (rng.standard_normal((DO, 8)) * 0.02).astype(np.float32)
    out = kernel(x, ids, W, b, lora_A, lora_B)
    print(out.shape, out.dtype, float(np.abs(out).mean()))
